# revision 48
# baseline (speedup 1.0000x reference)
"""Qwen3.5 vision attention (S=4096, H=16x80, block-diag mask) on 8 trn2 cores.

Sharding: tensor-parallel over heads (2 heads/core). Each core computes
qkv projection for its heads, rotary, block-sparse attention, and a partial
output projection (RowParallel); the host sums the 8 partials (all-reduce)
and adds proj_b.

v2 changes vs baseline (390 us):
  - bf16 for X/W/trig/q/k/v/probs/proj-weights: halves DMA and doubles
    DVE throughput; transposes run at 1 cyc/row instead of 2.
  - softmax normalization without the DRAM round-trip: denominator row is
    copied psum->sbuf (scalar), broadcast across 80 partitions with a
    rank-1 matmul (ones[1,80]^T @ denom[1,512]), and inverted with the
    fast custom-DVE reciprocal on the broadcast tile. This removes the
    16 x ~6us tensor-engine stalls that kept resetting the PE clock to
    its cold (half-rate) state.
  - ones column of V is memset on-chip instead of 4096 tiny DMA packets.
  - PSUM: pv double-buffered (transposes share the qkv accumulation tag)
    so PV of the next chunk overlaps normalization of the previous.
  - output staged per (block, row-chunk) as [128, 1024] bf16 and stored
    with one DMA each (40 stores instead of 80, half the bytes).
"""

import os
from contextlib import ExitStack

import numpy as np

S = 4096
HID = 1280
D = 80
NB = 4
BS = 1024
NHL = 2  # heads per core
NCORES = 8
SCALING = float(D) ** -0.5
NEG_THRESH = -1e8

_CACHE = {}


def _build(allowed, mask_add, qkv_dt_name="bfloat16", attn_dt_name="bfloat16",
           out_dt_name="bfloat16"):
    """Build + compile the per-core bass module.

    allowed: tuple over qb of tuple of kb blocks attended to.
    mask_add: frozenset of (qb, kb) needing an additive mask tile.
    """
    import concourse.bass as bass
    import concourse.mybir as mybir
    import concourse.tile as tile
    from concourse import bacc
    from concourse.masks import make_identity

    f32 = mybir.dt.float32
    f32r = mybir.dt.float32r
    dt_qkv = getattr(mybir.dt, qkv_dt_name)
    dt_a = getattr(mybir.dt, attn_dt_name)
    out_dt = getattr(mybir.dt, out_dt_name)
    use_mask = len(mask_add) > 0

    nc = bacc.Bacc(
        "TRN2", target_bir_lowering=False, debug=False, num_devices=NCORES
    )
    xt = nc.dram_tensor("xt", [HID, S], dt_qkv, kind="ExternalInput").ap()
    wt = nc.dram_tensor("wt", [HID, 480], dt_qkv, kind="ExternalInput").ap()
    bqkv = nc.dram_tensor("bqkv", [1, 480], f32, kind="ExternalInput").ap()
    cosd = nc.dram_tensor("cosd", [S, D], dt_a, kind="ExternalInput").ap()
    sind = nc.dram_tensor("sind", [S, D], dt_a, kind="ExternalInput").ap()
    pw = nc.dram_tensor("pw", [2, D, HID], dt_a, kind="ExternalInput").ap()
    onesd = nc.dram_tensor("onesd", [1, D], f32r, kind="ExternalInput").ap()
    if use_mask:
        maskt = nc.dram_tensor("maskt", [S, S], f32, kind="ExternalInput").ap()
    outp = nc.dram_tensor("outp", [HID, S], out_dt, kind="ExternalOutput").ap()

    EXP = mybir.ActivationFunctionType.Exp
    interleave = all(tuple(allowed[b]) == (b,) for b in range(NB))

    with ExitStack() as ctx:
        tc = ctx.enter_context(tile.TileContext(nc))

        # ---- constants ----
        cpool = ctx.enter_context(tc.tile_pool(name="cpool", bufs=1))
        wt_sb = cpool.tile([128, 10, 480], dt_qkv, tag="wt_sb", name="wt_sb")
        nc.sync.dma_start(out=wt_sb, in_=wt.rearrange("(kk p) c -> p kk c", p=128))
        bias_bc = cpool.tile([128, 480], f32, tag="bias_bc", name="bias_bc")
        nc.sync.dma_start(out=bias_bc, in_=bqkv[0:1, :].to_broadcast((128, 480)))
        ident = cpool.tile([128, 128], dt_a, tag="ident", name="ident")
        make_identity(nc, ident)
        pw_sb = cpool.tile([D, 2, HID], dt_a, tag="pw_sb", name="pw_sb")
        ones80 = cpool.tile([1, D], f32r, tag="ones80", name="ones80")

        def c_const():
            # deferred: not needed until the first attention/proj phase
            nc.sync.dma_start(out=pw_sb, in_=pw.rearrange("h d o -> d h o"))
            nc.sync.dma_start(out=ones80, in_=onesd)

        kT_sb = [
            cpool.tile([D, S], dt_a, tag=f"kT{h}_sb", name=f"kT{h}_sb")
            for h in range(NHL)
        ]

        # ---- pools ----
        xtp = ctx.enter_context(tc.tile_pool(name="xtp", bufs=20))
        trig = ctx.enter_context(tc.tile_pool(name="trig", bufs=2))
        qkp = ctx.enter_context(tc.tile_pool(name="qkp", bufs=2))
        t2p = ctx.enter_context(tc.tile_pool(name="t2p", bufs=2))
        vp = ctx.enter_context(tc.tile_pool(name="vp", bufs=1))
        qtp = ctx.enter_context(
            tc.tile_pool(name="qtp", bufs=4 if interleave else 8)
        )
        expp = ctx.enter_context(tc.tile_pool(name="expp", bufs=3))
        otp = ctx.enter_context(tc.tile_pool(name="otp", bufs=4))
        ddp = ctx.enter_context(tc.tile_pool(name="ddp", bufs=2))
        rbp = ctx.enter_context(tc.tile_pool(name="rbp", bufs=2))
        stg = ctx.enter_context(tc.tile_pool(name="stg", bufs=20))
        if use_mask:
            mtp = ctx.enter_context(tc.tile_pool(name="mtp", bufs=4))

        # PSUM: 8 banks total. qkv accumulation and the rotary transposes
        # share one tag (they alternate within the qkv phase).
        qkvps = ctx.enter_context(tc.tile_pool(name="qkvps", bufs=2, space="PSUM"))
        scps = ctx.enter_context(tc.tile_pool(name="scps", bufs=2, space="PSUM"))
        pvps = ctx.enter_context(tc.tile_pool(name="pvps", bufs=2, space="PSUM"))
        prps = ctx.enter_context(tc.tile_pool(name="prps", bufs=2, space="PSUM"))

        # PE warmup: the HAM clock gate only releases full rate after ~3.4us
        # of sustained matmul activity. The first ~14us of the kernel are
        # DMA-bound (weights + first block of X), so run throwaway matmuls
        # on a zeroed tile to arrive at the first real matmul already warm.
        scratch = cpool.tile([128, 128], dt_a, tag="scratch", name="scratch")
        nc.gpsimd.memset(scratch, 0.0)
        for w in range(60):
            wp = scps.tile([128, 128], f32, tag="sc", name=f"warm_{w}")
            nc.tensor.matmul(out=wp, lhsT=scratch, rhs=scratch)

        v_tiles = {}
        qT_tiles = {}

        def qkv_chunks(b):
            """Emission chunks for the qkv+rotary+transpose phase of block b.

            Returned as closures so they can be zipped between the previous
            block's attention chunks: alternating bass_priority makes the
            tile scheduler fill the exp-bound attention stretches with qkv
            matmuls.
            """
            st = {}

            def c_load_x():
                xts = []
                for k in range(10):
                    xt_t = xtp.tile([128, BS], dt_qkv, tag="xt", name=f"xt_{b}_{k}")
                    nc.sync.dma_start(
                        out=xt_t,
                        in_=xt[k * 128 : (k + 1) * 128, b * BS : (b + 1) * BS],
                    )
                    xts.append(xt_t)
                st["xts"] = xts

            def c_load():
                cos_t = trig.tile([128, 8, D], dt_a, tag="cos", name=f"cos_{b}")
                nc.sync.dma_start(
                    out=cos_t,
                    in_=cosd[b * BS : (b + 1) * BS, :].rearrange(
                        "(c p) d -> p c d", p=128
                    ),
                )
                sin_t = trig.tile([128, 8, D], dt_a, tag="sin", name=f"sin_{b}")
                nc.sync.dma_start(
                    out=sin_t,
                    in_=sind[b * BS : (b + 1) * BS, :].rearrange(
                        "(c p) d -> p c d", p=128
                    ),
                )
                st["cos"] = cos_t
                st["sin"] = sin_t
                st["qk"] = qkp.tile([128, 8, 320], dt_a, tag="qk", name=f"qk_{b}")

            def c_mm(m):
                def run():
                    qk_blk = st["qk"]
                    ps = qkvps.tile(
                        [128, 480], f32, tag="qkvps", name=f"qkvps_{b}_{m}"
                    )
                    for k in range(10):
                        nc.tensor.matmul(
                            out=ps,
                            lhsT=st["xts"][k][:, m * 128 : (m + 1) * 128],
                            rhs=wt_sb[:, k, :],
                            start=(k == 0),
                            stop=(k == 9),
                        )
                    nc.vector.tensor_add(
                        out=qk_blk[:, m, :], in0=ps[:, 0:320], in1=bias_bc[:, 0:320]
                    )
                    v_t = vp.tile(
                        [128, 2, 97], dt_a, tag="v", name=f"v_{b}_{m}",
                        bufs=16 if interleave else 32,
                    )
                    nc.vector.tensor_add(
                        out=v_t[:, :, 0:D],
                        in0=ps[:, 320:480].rearrange("p (h d) -> p h d", h=2),
                        in1=bias_bc[:, 320:480].rearrange("p (h d) -> p h d", h=2),
                    )
                    nc.gpsimd.memset(v_t[:, :, D:97], 1.0)
                    v_tiles[(b, m)] = v_t

                return run

            def c_rot(tau, h):
                def run():
                    qk_blk = st["qk"]
                    sin_t = st["sin"]
                    cos_t = st["cos"]
                    base = tau * 160 + h * D
                    sl = qk_blk[:, :, base : base + D]
                    t2 = t2p.tile(
                        [128, 8, D], dt_a, tag="t2", name=f"t2_{b}_{tau}_{h}"
                    )
                    nc.vector.tensor_mul(
                        out=t2[:, :, 0:40],
                        in0=qk_blk[:, :, base + 40 : base + D],
                        in1=sin_t[:, :, 0:40],
                    )
                    nc.vector.tensor_mul(
                        out=t2[:, :, 40:D],
                        in0=qk_blk[:, :, base : base + 40],
                        in1=sin_t[:, :, 40:D],
                    )
                    nc.vector.tensor_mul(out=sl, in0=sl, in1=cos_t)
                    nc.vector.tensor_add(out=sl, in0=sl, in1=t2)

                    if tau == 0:
                        dst_t = qtp.tile([D, BS], dt_a, tag="qt", name=f"qT_{b}_{h}")
                        qT_tiles[(b, h)] = dst_t
                    for g in range(2):
                        tp = qkvps.tile(
                            [D, 512], dt_a, tag="qkvps", name=f"tr_{b}_{tau}_{h}_{g}"
                        )
                        for j in range(4):
                            m = g * 4 + j
                            nc.tensor.matmul(
                                out=tp[:, j * 128 : (j + 1) * 128],
                                lhsT=qk_blk[:, m, base : base + D],
                                rhs=ident,
                                is_transpose=True,
                                start=(j == 0),
                                stop=(j == 3),
                            )
                        if tau == 0:
                            dst = qT_tiles[(b, h)][:, g * 512 : (g + 1) * 512]
                        else:
                            dst = kT_sb[h][
                                :, b * BS + g * 512 : b * BS + (g + 1) * 512
                            ]
                        nc.vector.tensor_copy(out=dst, in_=tp)

                return run

            chunks = [c_load_x, c_load]
            chunks += [c_mm(m) for m in range(8)]
            chunks += [c_rot(tau, h) for tau in range(2) for h in range(NHL)]
            return chunks

        def attn_chunks(b):
            kbs = allowed[b]
            nmm = len(kbs) * 8
            st = {}

            def c_alloc():
                st["ots"] = [
                    otp.tile([D, BS], dt_a, tag="ot", name=f"ot_{b}_{h}")
                    for h in range(NHL)
                ]
                st["sts"] = [
                    stg.tile([128, 2, 512], out_dt, tag="st", name=f"st_{b}_{o}")
                    for o in range(10)
                ]

            def c_attn(sh, h):
                def run():
                    qT_t = qT_tiles[(b, h)]
                    ot_t = st["ots"][h]
                    pv = pvps.tile(
                        [97, 512], f32, tag="pv", name=f"pv_{b}_{h}_{sh}"
                    )
                    i = 0
                    for kb in kbs:
                        for t in range(8):
                            scp = scps.tile(
                                [128, 512], f32, tag="sc",
                                name=f"sc_{b}_{h}_{sh}_{kb}_{t}",
                            )
                            nc.tensor.matmul(
                                out=scp,
                                lhsT=kT_sb[h][
                                    :, kb * BS + t * 128 : kb * BS + (t + 1) * 128
                                ],
                                rhs=qT_t[:, sh * 512 : (sh + 1) * 512],
                                start=True,
                                stop=True,
                            )
                            if (b, kb) in mask_add:
                                mk = mtp.tile(
                                    [128, 512], f32, tag="mk",
                                    name=f"mk_{b}_{h}_{sh}_{kb}_{t}",
                                )
                                nc.sync.dma_start(
                                    out=mk,
                                    in_=maskt[
                                        kb * BS + t * 128 : kb * BS + (t + 1) * 128,
                                        b * BS + sh * 512 : b * BS + (sh + 1) * 512,
                                    ],
                                )
                                nc.vector.tensor_add(out=scp, in0=scp, in1=mk)
                            ep = expp.tile(
                                [128, 512], dt_a, tag="exp",
                                name=f"ep_{b}_{h}_{sh}_{kb}_{t}",
                            )
                            nc.scalar.activation(
                                out=ep, in_=scp, func=EXP, scale=SCALING
                            )
                            nc.tensor.matmul(
                                out=pv,
                                lhsT=v_tiles[(kb, t)][:, h, :],
                                rhs=ep,
                                start=(i == 0),
                                stop=(i == nmm - 1),
                            )
                            i += 1
                    # normalize: dd = denom row (scalar copy, psum->sbuf),
                    # rb_ps = ones80^T @ dd (PE broadcast over partitions),
                    # rb = 1/rb_ps (fast custom-DVE reciprocal),
                    # ot = pv * rb.
                    dd = ddp.tile(
                        [1, 512], f32r, tag="dd", name=f"dd_{b}_{h}_{sh}"
                    )
                    nc.scalar.copy(out=dd, in_=pv[96:97, :])
                    rb_ps = prps.tile(
                        [D, 512], f32, tag="pr", name=f"rbps_{b}_{h}_{sh}"
                    )
                    nc.tensor.matmul(out=rb_ps, lhsT=ones80, rhs=dd)
                    rb = rbp.tile([D, 512], f32, tag="rb", name=f"rb_{b}_{h}_{sh}")
                    nc.vector.reciprocal_approx_fast(out=rb, in_=rb_ps)
                    nc.vector.tensor_mul(
                        out=ot_t[:, sh * 512 : (sh + 1) * 512],
                        in0=pv[0:D, :],
                        in1=rb,
                    )

                return run

            def c_proj(sh):
                def run():
                    ots = st["ots"]
                    sts = st["sts"]
                    for o in range(10):
                        pp = prps.tile(
                            [128, 512], f32, tag="pr", name=f"pr_{b}_{sh}_{o}"
                        )
                        nc.tensor.matmul(
                            out=pp,
                            lhsT=pw_sb[:, 0, o * 128 : (o + 1) * 128],
                            rhs=ots[0][:, sh * 512 : (sh + 1) * 512],
                            start=True,
                            stop=False,
                        )
                        nc.tensor.matmul(
                            out=pp,
                            lhsT=pw_sb[:, 1, o * 128 : (o + 1) * 128],
                            rhs=ots[1][:, sh * 512 : (sh + 1) * 512],
                            start=False,
                            stop=True,
                        )
                        nc.vector.tensor_copy(out=sts[o][:, sh, :], in_=pp)
                        if b == NB - 1:
                            eng = (nc.sync, nc.gpsimd, nc.scalar)[o % 3]
                        else:
                            eng = nc.gpsimd if o % 2 else nc.sync
                        eng.dma_start(
                            out=outp[
                                o * 128 : (o + 1) * 128,
                                b * BS + sh * 512 : b * BS + (sh + 1) * 512,
                            ],
                            in_=sts[o][:, sh, :],
                        )

                return run

            chunks = [c_alloc]
            for sh in range(2):
                chunks += [c_attn(sh, h) for h in range(NHL)]
                chunks.append(c_proj(sh))
            return chunks

        if interleave:
            # Software-pipelined emission: attention chunks of block b are
            # zipped with qkv chunks of block b+1 so the scheduler always
            # has dense tensor work during the exp-bound attention phase.
            first = qkv_chunks(0)
            first[0]()
            c_const()
            for c in first[1:]:
                c()
            carry = None
            for b in range(NB):
                at = attn_chunks(b)
                if carry is not None:
                    # block NB-2's last proj: fills the final block's
                    # attention bubbles (no next-block qkv remains)
                    at.insert(1, carry)
                    carry = None
                if b == NB - 2:
                    carry = at.pop()
                nxt = qkv_chunks(b + 1) if b + 1 < NB else []
                k = 0
                for i, a in enumerate(at):
                    a()
                    take = (len(nxt) * (i + 1)) // len(at) - k
                    for _ in range(take):
                        nxt[k]()
                        k += 1
        else:
            c_const()
            for b in range(NB):
                for c in qkv_chunks(b):
                    c()
            for b in range(NB):
                for c in attn_chunks(b):
                    c()

    nc.compile()
    return nc


def _analyze_mask(mask):
    m = np.asarray(mask).reshape(S, S)
    allowed = []
    mask_add = set()
    for qb in range(NB):
        row = []
        for kb in range(NB):
            t = m[qb * BS : (qb + 1) * BS, kb * BS : (kb + 1) * BS]
            if np.all(t <= NEG_THRESH):
                continue
            row.append(kb)
            if not np.all(t == 0.0):
                mask_add.add((qb, kb))
        if not row:
            raise NotImplementedError("fully masked query block")
        allowed.append(tuple(row))
    return tuple(allowed), frozenset(mask_add)


def _np_dt(name):
    if name == "bfloat16":
        import ml_dtypes

        return ml_dtypes.bfloat16
    return np.float32


def kernel(
    hidden_states, attention_mask, cos, sin, qkv_w, qkv_b, proj_w, proj_b
):
    from concourse import bass_utils

    qkv_dt = os.environ.get("KERNEL_QKV_DT", "bfloat16")
    attn_dt = os.environ.get("KERNEL_ATTN_DT", "bfloat16")
    out_dt = os.environ.get("KERNEL_OUT_DT", "bfloat16")
    trace = bool(int(os.environ.get("KERNEL_TRACE", "0")))

    X = np.ascontiguousarray(np.asarray(hidden_states, dtype=np.float32))
    allowed, mask_add = _analyze_mask(attention_mask)

    key = (qkv_dt, attn_dt, out_dt, allowed, mask_add)
    if key not in _CACHE:
        _CACHE[key] = _build(
            allowed, mask_add, qkv_dt_name=qkv_dt, attn_dt_name=attn_dt,
            out_dt_name=out_dt,
        )
    nc = _CACHE[key]

    np_qkv = _np_dt(qkv_dt)
    np_attn = _np_dt(attn_dt)

    XT = np.ascontiguousarray(X.T).astype(np_qkv)
    cos = np.ascontiguousarray(np.asarray(cos, dtype=np.float32))
    sin = np.asarray(sin, dtype=np.float32)
    sinh = np.ascontiguousarray(
        np.concatenate([-sin[:, : D // 2], sin[:, D // 2 :]], axis=1)
    )
    qkv_w = np.asarray(qkv_w, dtype=np.float32)
    qkv_b = np.asarray(qkv_b, dtype=np.float32)
    proj_w = np.asarray(proj_w, dtype=np.float32)
    proj_b = np.asarray(proj_b, dtype=np.float32)

    in_maps = []
    for c in range(NCORES):
        j0 = c * NHL * D
        sl = slice(j0, j0 + NHL * D)
        Wc = np.concatenate(
            [qkv_w[sl], qkv_w[HID:][sl], qkv_w[2 * HID :][sl]], axis=0
        )
        m = {
            "xt": XT,
            "wt": np.ascontiguousarray(Wc.T).astype(np_qkv),
            "bqkv": np.ascontiguousarray(
                np.concatenate([qkv_b[sl], qkv_b[HID:][sl], qkv_b[2 * HID :][sl]])[
                    None, :
                ]
            ),
            "onesd": np.ones((1, D), dtype=np.float32),
            "cosd": cos.astype(np_attn),
            "sind": sinh.astype(np_attn),
            "pw": np.ascontiguousarray(
                np.stack(
                    [
                        proj_w[:, j0 : j0 + D].T,
                        proj_w[:, j0 + D : j0 + 2 * D].T,
                    ]
                )
            ).astype(np_attn),
        }
        if mask_add:
            m["maskt"] = np.ascontiguousarray(
                (np.asarray(attention_mask).reshape(S, S).T / SCALING).astype(
                    np.float32
                )
            )
        in_maps.append(m)

    res = bass_utils.run_bass_kernel_spmd(
        nc, in_maps, core_ids=list(range(NCORES)), trace=trace
    )
    global LAST_RESULT
    LAST_RESULT = res

    acc = np.zeros((HID, S), dtype=np.float64)
    for c in range(NCORES):
        acc += np.asarray(res.results[c]["outp"], dtype=np.float64)
    out = acc.T + proj_b.astype(np.float64)[None, :]
    return out.astype(np.float32)


LAST_RESULT = None
